# revision 14
# baseline (speedup 1.0000x reference)
"""Trainium2 Bass kernel for BasicAttention with softmax over the QUERY axis.

reference:
    scores = einsum("bqd,bkd->bqk", q, k)      # [B,Q,K]
    attn   = softmax(scores, axis=1)           # over q (per (b,k) column)
    out    = einsum("bqk,bkd->bqd", attn, v)   # [B,Q,D]

Shapes: B=8, Q=K=2048, D=1024, fp32.

Strategy: batch-parallel over the 8 NeuronCores (one batch element per
core). Per core everything is phrased in the transposed score layout
scoresT[k, q] so the softmax reduction runs along the free axis, and the
attn block feeding the second matmul is already [k, q] = lhsT layout.

Normalization is folded into V: attnt stores the un-normalized exp(s-m)
(m is per-k-column so it cancels exactly), and each V row k is scaled by
rz[k] = 1/sum_q exp(s-m) instead - same result, one [128,1024] scale
instead of a [128,2048] one, and phase 2 no longer waits on it.

Pipelining: Q and K stream in on both HWDGE queues (SP + Act) in 2-tile
groups; the first scores matmul fires as soon as q[0:512] and K tile 0
are transposed (~7us) while the rest of Q is still in flight. Transposes
run in f32r (1.5 cycles/row vs 2.0 for fp32), packed 4 per PSUM bank
with single [128,512] copies, placed on the otherwise-idle engines ahead
of the exp work so the next tile's matmuls never wait. V is DMA'd with
an in-flight fp32->bf16 cast by the gpsimd SWDGE.
"""

import sys

sys.path.insert(0, "/opt/trn_rl_repo")

from contextlib import ExitStack

import numpy as np

import concourse.bass as bass
import concourse.tile as tile
from concourse import bacc, bass_utils, mybir

B, NQ, NK, D = 8, 2048, 2048, 1024
P = 128                 # partition size
DC = D // P             # 8 d-chunks
KT_N = NK // P          # 16 k-tiles
QT_N = NQ // P          # 16 q-tiles
NC_ = 512               # softmax chunk (one PSUM bank of fp32)

F32 = mybir.dt.float32
F32R = mybir.dt.float32r
BF16 = mybir.dt.bfloat16

_cached = None


def _build():
    nc = bacc.Bacc("TRN2", debug=False, num_devices=B)

    q_dram = nc.dram_tensor("q", (NQ, D), F32R, kind="ExternalInput").ap()
    k_dram = nc.dram_tensor("k", (NK, D), F32R, kind="ExternalInput").ap()
    v_dram = nc.dram_tensor("v", (NK, D), F32, kind="ExternalInput").ap()
    id_dram = nc.dram_tensor("ident", (P, P), F32R, kind="ExternalInput").ap()
    out_dram = nc.dram_tensor("out", (NQ, D), F32, kind="ExternalOutput").ap()

    with tile.TileContext(nc) as tc:
        with ExitStack() as ctx:
            big_pool = ctx.enter_context(tc.tile_pool(name="big", bufs=1))
            const_pool = ctx.enter_context(tc.tile_pool(name="const", bufs=1))
            small_pool = ctx.enter_context(tc.tile_pool(name="small", bufs=4))

            ident = const_pool.tile([P, P], F32R)
            nc.sync.dma_start(ident[:], id_dram[:])

            # persistent big tensors
            qt = big_pool.tile([P, DC * NQ], F32R, tag="qt")       # 64 KB/part
            attnt = big_pool.tile([P, KT_N * NQ], BF16, tag="at")  # 64 KB/part
            vt = big_pool.tile([P, KT_N * D], BF16, tag="vt")      # 32 KB/part
            qt_v = qt[:].rearrange("p (dc q) -> p dc q", dc=DC)

            ph1_ctx = ExitStack()
            qnat_pool = ph1_ctx.enter_context(tc.tile_pool(name="qnat", bufs=4))
            knat_pool = ph1_ctx.enter_context(tc.tile_pool(name="knat", bufs=3))
            ktile_pool = ph1_ctx.enter_context(tc.tile_pool(name="ktp", bufs=2))
            vnat_pool = ph1_ctx.enter_context(tc.tile_pool(name="vnat", bufs=2))
            # tp pool first: its banks free earliest at the end of phase 1
            # (k15 needs no next-tile transposes), so phase 2's o_psum -
            # allocated from the same base after close - starts immediately.
            tp_psum = ph1_ctx.enter_context(
                tc.tile_pool(name="tpsum", bufs=3, space="PSUM")
            )
            sc_psum = ph1_ctx.enter_context(
                tc.tile_pool(name="spsum", bufs=5, space="PSUM")
            )

            # ---- DMA issue. Per-core HBM bandwidth (~330 GB/s) is shared
            # across all queues, so priority order is everything: Q first
            # (it gates every scores matmul), K0 early, then K/V streamed.
            qts = []
            kgs = []

            def q_dma(t, eng):
                qg = qnat_pool.tile([P, D], F32R, tag="qg")
                eng.dma_start(qg[:], q_dram[t * P:(t + 1) * P, :])
                qts.append(qg)

            def k_dma(kt, eng):
                kg = knat_pool.tile([P, D], F32R, tag="kg")
                eng.dma_start(kg[:], k_dram[kt * P:(kt + 1) * P, :])
                kgs.append(kg)

            vnats = []

            def v_dma(kt, eng):
                vn = vnat_pool.tile([P, D], F32, tag="vn")
                eng.dma_start(vn[:], v_dram[kt * P:(kt + 1) * P, :])
                vnats.append(vn)

            k_dma(0, nc.scalar)
            for t in range(QT_N):
                q_dma(t, nc.sync if t % 2 == 0 else nc.scalar)
            # K and V interleaved behind Q on both rings; the in-order DMA
            # rings guarantee Q gets the full HBM bandwidth first, and the
            # vnat buffer rotation paces V to ~1 tile per k-iteration.
            for kt in range(1, KT_N):
                k_dma(kt, nc.scalar if kt % 2 == 0 else nc.sync)
                v_dma(kt - 1, nc.sync if kt % 2 == 0 else nc.scalar)
            v_dma(KT_N - 1, nc.scalar)

            # ---- transpose helpers (PE; packed 4 per PSUM bank) ----
            copy_flip = [0]

            def pack_copy(dst_ap, pt):
                # alternate the [128,512] PSUM->SBUF copies between the
                # two lighter engines; they run ahead of the exp work.
                if copy_flip[0] % 2 == 0:
                    nc.scalar.copy(dst_ap, pt[:])
                else:
                    nc.vector.tensor_copy(dst_ap, pt[:])
                copy_flip[0] += 1

            def dummy_mm():
                # tiny real matmul keeps the HAM clock ramped while the
                # PE is doing transpose-only stretches (output unread)
                dm = sc_psum.tile([P, NC_], F32, tag="sc")
                nc.tensor.matmul(dm[:, 0:64], ident[:], ident[:, 0:64],
                                 start=True, stop=True)

            def tp_q_tile(t):
                # 8 transposes -> 2 packs -> 2 copies into qt
                qg = qts[t]
                dummy_mm()
                for pp in range(2):
                    pt = tp_psum.tile([P, NC_], F32R, tag="tp")
                    for j in range(4):
                        dc = pp * 4 + j
                        nc.tensor.transpose(
                            pt[:, j * P:(j + 1) * P],
                            qg[:, dc * P:(dc + 1) * P],
                            ident[:],
                        )
                    dst = qt_v[:, pp * 4:pp * 4 + 4, t * P:(t + 1) * P]
                    pack_copy(dst, pt)

            def tp_k_tile(kt):
                # 8 transposes -> 2 packs -> ktile [128, 1024] f32r
                kg = kgs[kt]
                ktile = ktile_pool.tile([P, D], F32R, tag="kt")
                dummy_mm()
                for pp in range(2):
                    pt = tp_psum.tile([P, NC_], F32R, tag="tp")
                    for j in range(4):
                        dc = pp * 4 + j
                        nc.tensor.transpose(
                            pt[:, j * P:(j + 1) * P],
                            kg[:, dc * P:(dc + 1) * P],
                            ident[:],
                        )
                    pack_copy(ktile[:, pp * NC_:(pp + 1) * NC_], pt)
                return ktile

            # ---- per-k-tile scores + softmax (4 chunks of 512 q) ----
            def scores_chunk(ktile, kt, c):
                sc = sc_psum.tile([P, NC_], F32, tag="sc")
                for dc in range(DC):
                    nc.tensor.matmul(
                        sc[:],
                        ktile[:, dc * P:(dc + 1) * P],
                        qt[:, dc * NQ + c * NC_: dc * NQ + (c + 1) * NC_],
                        start=(dc == 0),
                        stop=(dc == DC - 1),
                    )
                nm = small_pool.tile([P, 1], F32, tag=f"nm{c}")
                nc.vector.reduce_max(
                    nm[:], sc[:], axis=mybir.AxisListType.X, negate=True
                )
                return sc, nm

            def softmax_tile(kt, scs, nms):
                m01 = small_pool.tile([P, 1], F32, tag="m01")
                m23 = small_pool.tile([P, 1], F32, tag="m23")
                negmax = small_pool.tile([P, 1], F32, tag="nmg")
                nc.vector.tensor_tensor(
                    m01[:], nms[0][:], nms[1][:], op=mybir.AluOpType.min
                )
                nc.vector.tensor_tensor(
                    m23[:], nms[2][:], nms[3][:], op=mybir.AluOpType.min
                )
                nc.vector.tensor_tensor(
                    negmax[:], m01[:], m23[:], op=mybir.AluOpType.min
                )
                sums = []
                for c in range(4):
                    sm = small_pool.tile([P, 1], F32, tag=f"sm{c}")
                    nc.scalar.activation(
                        attnt[:, kt * NQ + c * NC_: kt * NQ + (c + 1) * NC_],
                        scs[c][:],
                        mybir.ActivationFunctionType.Exp,
                        bias=negmax[:], scale=1.0, accum_out=sm[:],
                    )
                    sums.append(sm)
                t01 = small_pool.tile([P, 1], F32, tag="t01")
                t23 = small_pool.tile([P, 1], F32, tag="t23")
                tot = small_pool.tile([P, 1], F32, tag="tot")
                rz = small_pool.tile([P, 1], F32, tag="rz")
                nc.vector.tensor_add(t01[:], sums[0][:], sums[1][:])
                nc.vector.tensor_add(t23[:], sums[2][:], sums[3][:])
                nc.vector.tensor_add(tot[:], t01[:], t23[:])
                nc.vector.reciprocal(rz[:], tot[:])
                # fused cast + normalization fold into V rows, on the
                # otherwise-idle gpsimd engine (per-k scale, exact)
                nc.gpsimd.tensor_scalar_mul(
                    vt[:, kt * D:(kt + 1) * D], vnats[kt][:], rz[:]
                )

            # ---- startup: interleave Q transposes with k-tile 0 ----
            for t in range(4):
                tp_q_tile(t)
            ktile0 = tp_k_tile(0)
            scs, nms = [], []
            for c in range(4):
                sc, nm = scores_chunk(ktile0, 0, c)
                scs.append(sc)
                nms.append(nm)
                if c < 3:
                    for t in range(4 * (c + 1), 4 * (c + 2)):
                        tp_q_tile(t)
            ktile_cur = tp_k_tile(1)
            softmax_tile(0, scs, nms)

            # ---- main k-loop ----
            for kt in range(1, KT_N):
                scs, nms = [], []
                ktile_next = None
                for c in range(4):
                    sc, nm = scores_chunk(ktile_cur, kt, c)
                    scs.append(sc)
                    nms.append(nm)
                    if c == 0 and kt + 1 < KT_N:
                        ktile_next = tp_k_tile(kt + 1)
                softmax_tile(kt, scs, nms)
                ktile_cur = ktile_next

            # ---- phase 2: out[q, d] = sum_kt attnT[kt].T @ (V[kt]*rz) ----
            ph1_ctx.close()
            # bufs=3: lands exactly on the freed tp banks (0-2), which have
            # no phase-1 readers left, so the first out chain starts without
            # waiting for k15's exp to release a score bank.
            o_psum = ctx.enter_context(
                tc.tile_pool(name="opsum", bufs=3, space="PSUM")
            )
            out_pool = ctx.enter_context(tc.tile_pool(name="outp", bufs=4))
            for qt_i in range(QT_N):
                for dt_i in range(2):
                    po = o_psum.tile([P, NC_], F32, tag="po")
                    for kt in range(KT_N):
                        nc.tensor.matmul(
                            po[:],
                            attnt[:, kt * NQ + qt_i * P: kt * NQ + (qt_i + 1) * P],
                            vt[:, kt * D + dt_i * NC_: kt * D + (dt_i + 1) * NC_],
                            start=(kt == 0),
                            stop=(kt == KT_N - 1),
                        )
                    osb = out_pool.tile([P, NC_], F32, tag="ot")
                    if dt_i == 0:
                        nc.vector.tensor_copy(osb[:], po[:])
                    else:
                        nc.scalar.copy(osb[:], po[:])
                    (nc.sync if dt_i == 0 else nc.scalar).dma_start(
                        out_dram[qt_i * P:(qt_i + 1) * P,
                                 dt_i * NC_:(dt_i + 1) * NC_],
                        osb[:],
                    )

    nc.compile()
    return nc


def _get_module():
    global _cached
    if _cached is None:
        _cached = _build()
    return _cached


_IDENT = np.eye(P, dtype=np.float32)


def run(queries, keys, values, trace=False, trace_kwargs=None):
    """Run on 8 cores; returns (output [B,NQ,D] fp32, BassKernelResults)."""
    queries = np.ascontiguousarray(np.asarray(queries, dtype=np.float32))
    keys = np.ascontiguousarray(np.asarray(keys, dtype=np.float32))
    values = np.ascontiguousarray(np.asarray(values, dtype=np.float32))
    assert queries.shape == (B, NQ, D), queries.shape

    nc = _get_module()
    in_maps = [
        {"q": queries[b], "k": keys[b], "v": values[b], "ident": _IDENT}
        for b in range(B)
    ]
    res = bass_utils.run_bass_kernel_spmd(
        nc, in_maps, core_ids=list(range(B)), trace=trace,
        **(trace_kwargs or {}),
    )
    out = np.stack([res.results[b]["out"] for b in range(B)], axis=0)
    return out, res


def kernel(queries, keys, values):
    out, _ = run(queries, keys, values)
    return out
